# revision 2
# baseline (speedup 1.0000x reference)
"""Corr2Cost sampling kernel for 8 TRN2 NeuronCores.

Math: out[b,c,k,i,j] = lerp of corr[b,c,:,i,j] at depth (j + k - maxdisp)
(is_ux=1) with zero padding outside [0, D-1].  For integer maxdisp the
displacements linspace(-md, md, 2*md+1) are exact integers, so the lerp
weight is exactly 0 and the op is a pure masked integer gather:

    out[b,c,k,i,j] = corr[b,c, j+k-md, i, j]   if 0 <= j+k-md < D else 0

Sharding: data-parallel over the 16 (b,c) pairs -> 2 pairs per core; no
cross-core communication.

Layout strategy (all DMA at max descriptor efficiency):
  - host pre-transposes corr pair slabs to (i, d, j) so the per-pair load
    is 96 partitions x 64KB contiguous;
  - on chip, A[i, d*W + j]; the gather for output row k is the constant-
    stride-(W+1) slice A[i, j*(W+1) + (k-md)*W] -> one strided
    tensor_copy per k into O[i, kk*W + j];
  - O is stored as (i, k, j) slabs (per-partition contiguous), host
    post-transposes to (k, i, j).  Border regions (|j - valid| outside
    range) are zeroed by 4 rectangular over-memsets per pair.
"""

import numpy as np

B, C, D, H, W = 8, 2, 128, 96, 128
N_CORES = 8
PAIRS = B * C  # 16
PAIRS_PER_CORE = PAIRS // N_CORES  # 2

_NC_CACHE = {}


def _k_chunks(K):
    """Split [0, K) into 4 nearly equal chunks."""
    n = 4 if K >= 8 else 1
    bounds = [round(i * K / n) for i in range(n + 1)]
    return [(bounds[i], bounds[i + 1]) for i in range(n)]


def _build_bass(md: int, reps: int = 1):
    """Build + compile the per-core Bass graph for is_ux=1, given maxdisp.

    reps > 1 wraps the body in a hardware For_i loop (timing harness only).
    """
    import concourse.bacc as bacc
    import concourse.mybir as mybir
    import concourse.tile as tile

    K = 2 * md + 1
    f32 = mybir.dt.float32

    nc = bacc.Bacc("TRN2", target_bir_lowering=False, debug=False)
    x = nc.dram_tensor("x", [PAIRS_PER_CORE, H, D * W], f32, kind="ExternalInput")
    y = nc.dram_tensor("y", [PAIRS_PER_CORE, H, K * W], f32, kind="ExternalOutput")

    def body(tc, apool, opool):
        for p in range(PAIRS_PER_CORE):
            a = apool.tile([H, D * W], f32)
            nc.sync.dma_start(out=a[:], in_=x[p])
            for (k0, k1) in _k_chunks(K):
                ck = k1 - k0
                o = opool.tile([H, ck * W], f32)
                o3 = o[:].rearrange("p (kk j) -> p kk j", j=W)
                # rectangular over-memset covering the masked border of
                # this chunk (copies below overwrite the valid part)
                lmax = max(0, md - k0)          # left border width at k0
                rmax = max(0, k1 - 1 - md)      # right border width at k1-1
                if lmax > 0:
                    nc.vector.memset(o3[:, :, 0:lmax], 0.0)
                if rmax > 0:
                    nc.vector.memset(o3[:, :, W - rmax : W], 0.0)
                for k in range(k0, k1):
                    j0 = max(0, md - k)
                    j1 = min(W - 1, D - 1 + md - k)
                    n = j1 - j0 + 1
                    # d = j + k - md  ->  flat d*W + j = j*(W+1) + (k-md)*W
                    off0 = j0 * (W + 1) + (k - md) * W
                    kk = k - k0
                    nc.vector.tensor_copy(
                        o3[:, kk, j0 : j1 + 1],
                        a[:, off0 : off0 + (n - 1) * (W + 1) + 1 : W + 1],
                    )
                nc.sync.dma_start(out=y[p][:, k0 * W : k1 * W], in_=o[:])

    with tile.TileContext(nc) as tc:
        with (
            tc.tile_pool(name="a", bufs=2) as apool,
            tc.tile_pool(name="o", bufs=3) as opool,
        ):
            if reps == 1:
                body(tc, apool, opool)
            else:
                with tc.For_i(0, reps, 1):
                    body(tc, apool, opool)

    nc.compile()
    return nc


def _get_nc(md: int, reps: int = 1):
    key = (md, reps)
    if key not in _NC_CACHE:
        _NC_CACHE[key] = _build_bass(md, reps)
    return _NC_CACHE[key]


def _numpy_ref(corr, maxdisp, is_ux):
    """Exact numpy replication of the reference (fallback path)."""
    corr = np.asarray(corr)
    b, c, d_, h, w = corr.shape
    K = 2 * maxdisp + 1
    dx = np.linspace(-float(maxdisp), float(maxdisp), K).astype(np.float32)
    if is_ux:
        base = np.broadcast_to(np.arange(w, dtype=np.float32)[None, :], (h, w))
    else:
        base = np.broadcast_to(np.arange(h, dtype=np.float32)[:, None], (h, w))
    pos = base[None, :, :] + dx[:, None, None]
    i0f = np.floor(pos)
    w1 = (pos - i0f).astype(corr.dtype)
    i0 = i0f.astype(np.int32)
    i1 = i0 + 1
    m0 = ((i0 >= 0) & (i0 < d_)).astype(corr.dtype)
    m1 = ((i1 >= 0) & (i1 < d_)).astype(corr.dtype)
    idx0 = np.clip(i0, 0, d_ - 1)[None, None]
    idx1 = np.clip(i1, 0, d_ - 1)[None, None]
    g0 = np.take_along_axis(corr, np.broadcast_to(idx0, (b, c, K, h, w)), axis=2)
    g1 = np.take_along_axis(corr, np.broadcast_to(idx1, (b, c, K, h, w)), axis=2)
    return g0 * ((1.0 - w1) * m0)[None, None] + g1 * (w1 * m1)[None, None]


def _run_on_device(corr, md: int, reps: int = 1):
    from concourse.bass_utils import run_bass_kernel_spmd

    K = 2 * md + 1
    nc = _get_nc(md, reps)
    # (B, C, D, H, W) -> (16, D, H, W) -> (16, H, D, W) -> rows of D*W
    flat = np.asarray(corr).reshape(PAIRS, D, H, W)
    xt = np.ascontiguousarray(flat.transpose(0, 2, 1, 3)).reshape(PAIRS, H, D * W)
    in_maps = [
        {"x": xt[PAIRS_PER_CORE * c : PAIRS_PER_CORE * (c + 1)]}
        for c in range(N_CORES)
    ]
    res = run_bass_kernel_spmd(nc, in_maps, core_ids=list(range(N_CORES)))
    out = np.concatenate([res.results[c]["y"] for c in range(N_CORES)], axis=0)
    # (16, H, K*W) -> (16, H, K, W) -> (16, K, H, W) -> (B, C, K, H, W)
    out = out.reshape(PAIRS, H, K, W).transpose(0, 2, 1, 3)
    out = np.ascontiguousarray(out).reshape(B, C, K, H, W)
    return out, res


def kernel(corr, maxdisp, is_ux):
    corr = np.asarray(corr)
    md = int(maxdisp)
    ux = int(is_ux)
    if ux != 1 or md < 1 or md > 127 or corr.shape != (B, C, D, H, W):
        return _numpy_ref(corr, md, ux).astype(corr.dtype)
    out, _ = _run_on_device(corr, md)
    return out
